# revision 1
# baseline (speedup 1.0000x reference)
"""3-layer GCN + global mean pool + linear head on 8 Trainium2 NeuronCores.

Strategy (dst-sharded message passing):
  - GCN normalization factorizes: norm_e = dinv[src]*dinv[dst], so each conv
    layer is  h' = relu( dinv * ((Adj+I) @ (dinv * h)) @ W + b ).  Only pure
    row gather + segment-sum on device; diagonal scalings are per-node ops.
  - Nodes (and their in-edges, self-loops appended) are sharded across the 8
    cores by contiguous dst ranges.  Edges are grouped by (dst 128-block,
    source window) and padded to a uniform number of 128-edge chunks (padding
    uses dst_rel=-1, whose one-hot column is zero).
  - Per layer: each core row-scales its h slice (h~ = dinv*h, bf16, features
    padded to 128 so rows are 256B), slices are AllGathered into a DRAM
    table, and each core gathers its edges' source rows with dma_gather
    (int16 indices -> the table is split into 4 windows of 25088 rows; calls
    are <=1024 tokens to fit the SWDGE descriptor ring).  Segment-sum runs on
    the TensorEngine: per 128-edge chunk, aggT[64f,128d] += msgs[128e,0:64].T
    @ M[128e,128d], with M built by one broadcast is_equal against an iota
    tile.  The layer weight applies after aggregation (W commutes with the
    sum), then dinv/bias/relu as per-partition DVE ops.
  - Mean-pool uses the same one-hot matmul against graph ids, partials are
    AllReduced, and the head matmul finishes on every core.

Host-side work is sharding-style preprocessing only: edge sort/group/pad,
degree bincount (dinv), graph-size bincount, layout permutation.
"""
"""3-layer GCN + global mean pool + linear head on 8 Trainium2 NeuronCores.

Strategy (dst-sharded message passing):
  - GCN normalization factorizes: norm_e = dinv[src]*dinv[dst], so each conv
    layer is  h' = relu( dinv * ((Adj+I) @ (dinv * h)) @ W + b ).  Only pure
    row gather + segment-sum on device; diagonal scalings are per-node ops.
  - Nodes (and their in-edges, self-loops appended) are sharded across the 8
    cores by contiguous dst ranges.  Edges are grouped by (dst 128-block,
    source window) and padded to a uniform number of 128-edge chunks (padding
    uses dst_rel=-1, whose one-hot column is zero).
  - Per layer: each core row-scales its h slice (h~ = dinv*h, bf16, features
    padded to 128 so rows are 256B), slices are AllGathered into a DRAM
    table, and each core gathers its edges' source rows with dma_gather
    (int16 indices -> the table is split into 4 windows of 25088 rows; calls
    are <=1024 tokens to fit the SWDGE descriptor ring).  Segment-sum runs on
    the TensorEngine: per 128-edge chunk, aggT[64f,128d] += msgs[128e,0:64].T
    @ M[128e,128d], with M built by one broadcast is_equal against an iota
    tile.  The layer weight applies after aggregation (W commutes with the
    sum), then dinv/bias/relu as per-partition DVE ops.
  - Mean-pool uses the same one-hot matmul against graph ids, partials are
    AllReduced, and the head matmul finishes on every core.

Host-side work is sharding-style preprocessing only: edge sort/group/pad,
degree bincount (dinv), graph-size bincount, layout permutation.
"""
import numpy as np
import ml_dtypes

P = 128
NCORES = 8
NWIN = 4          # int16 source windows
TMAX = 1024       # max tokens per dma_gather call (SWDGE ring limit)
SGBLK = 7         # dst blocks per super-group (msgs buffer granularity)

# Full-size problem dims (nn_GCN_13881334300836)
N_FULL, E_FULL, D_FULL, C_FULL, G_FULL = 100_000, 1_250_000, 64, 10, 128


# --------------------------------------------------------------------------
# Host preprocessing
# --------------------------------------------------------------------------

def preprocess(x, edge_index, batch, n_cores=NCORES):
    """Shard nodes/edges; build window-grouped, chunk-padded gather indices.

    Permuted global row for node n (core, local=n-core*npc, b=local//P,
    p=local%P):  row = (core*P + p)*nblk + b.  Each core's h~ slice is then
    one contiguous SBUF->DRAM DMA and AllGather concatenation.  Window w
    covers rows [w*wrows, (w+1)*wrows), wrows = n_cores*P*nblk/NWIN.
    """
    N, H = x.shape
    assert N % n_cores == 0
    npc = N // n_cores
    nblk = (npc + P - 1) // P
    npad = nblk * P
    wrows = n_cores * npad // NWIN
    assert wrows <= 32768

    ei = edge_index.astype(np.int64)
    src_all = ei[0]
    dst_all = ei[1]

    # degree includes the self-loop (gcn_norm); loops are NOT streamed --
    # their term is one identity matmul per block on-chip.
    deg = (np.bincount(dst_all, minlength=N) + 1).astype(np.float32)
    dinv = (1.0 / np.sqrt(np.maximum(deg, 1.0))).astype(np.float32)

    # serpentine deal by in-degree -> balanced (core, slot) bins
    nbins = n_cores * nblk
    order = np.argsort(-deg, kind="stable")
    idx = np.arange(N)
    rr = idx // nbins
    kk = idx % nbins
    binidx = np.where(rr % 2 == 0, kk, nbins - 1 - kk)
    node_bin = np.empty(N, np.int64)
    node_lane = np.empty(N, np.int64)
    node_bin[order] = binidx
    node_lane[order] = rr
    core_of_n = node_bin % n_cores
    slot_of_n = node_bin // n_cores
    perm_n = (core_of_n * P + node_lane) * nblk + slot_of_n
    src_perm = perm_n[src_all]
    win_all = src_perm // wrows
    e_core = core_of_n[dst_all]
    e_slot = slot_of_n[dst_all]
    e_lane = node_lane[dst_all]

    # per-core (block, window) edge lists
    edges = []   # [core][b][w] -> (idx int16 array, drel array)
    K = np.zeros((n_cores, nblk, NWIN), np.int64)
    for c in range(n_cores):
        m = e_core == c
        s = src_perm[m]
        w = win_all[m]
        dslot = e_slot[m]
        dlane = e_lane[m]
        key = dslot * NWIN + w
        o2 = np.argsort(key, kind="stable")
        s, w, key = s[o2], w[o2], key[o2]
        dlane = dlane[o2]
        cnt = np.bincount(key, minlength=nblk * NWIN)
        off = np.zeros(nblk * NWIN + 1, np.int64)
        np.cumsum(cnt, out=off[1:])
        percore = []
        for b in range(nblk):
            row = []
            for ww in range(NWIN):
                k = b * NWIN + ww
                sl = slice(off[k], off[k + 1])
                row.append(((s[sl] - ww * wrows).astype(np.int16),
                            dlane[sl].astype(np.float32)))
                K[c, b, ww] = (cnt[k] + P - 1) // P
            percore.append(row)
        edges.append(percore)

    Kmax = K.max(axis=0)          # [nblk, NWIN] uniform chunk counts
    sgblk = SGBLK
    if nblk % sgblk != 0:
        sgblk = next((g for g in (7, 8, 6, 5, 4, 9, 10, 3, 2) if nblk % g == 0),
                     nblk)
    nsg = nblk // sgblk

    # token stream: for sg: for w: for b in sg: Kmax[b,w] chunks of 128
    # chunk positions (global column index) and per-(sg,w) call splits
    chunk_pos = np.zeros((nblk, NWIN), np.int64)   # starting chunk column
    sg_tok0 = []                                   # sg -> token start
    sg_w_ranges = []                               # sg -> [(w, tok0, tok1)]
    pos = 0
    for sg in range(nsg):
        sg_tok0.append(pos * P)
        rngs = []
        for ww in range(NWIN):
            t0 = pos * P
            for b in range(sg * sgblk, (sg + 1) * sgblk):
                chunk_pos[b, ww] = pos
                pos += Kmax[b, ww]
            rngs.append((ww, t0, pos * P))
        sg_w_ranges.append(rngs)
    nchunk = pos
    ntok = nchunk * P

    eidx16 = np.zeros((n_cores, 16, ntok // 16), np.int16)
    edst = np.full((n_cores, P, nchunk), -1.0, np.float32)
    for c in range(n_cores):
        stream = np.zeros(ntok, np.int16)
        for b in range(nblk):
            for ww in range(NWIN):
                s16, dr = edges[c][b][ww]
                t0 = chunk_pos[b, ww] * P
                stream[t0:t0 + len(s16)] = s16
                # dst_rel per slot: token t -> (p=t%128, chunk=t//128)
                col = chunk_pos[b, ww]
                nchunks_b = Kmax[b, ww]
                dcols = np.full((nchunks_b * P,), -1.0, np.float32)
                dcols[:len(dr)] = dr
                edst[c][:, col:col + nchunks_b] = dcols.reshape(nchunks_b, P).T
        eidx16[c] = stream.reshape(ntok // 16, 16).T

    dinv_pc = np.zeros((n_cores, P, nblk), np.float32)
    bat_pc = np.full((n_cores, P, nblk), -1.0, np.float32)
    xf = np.asarray(x, np.float32)
    xp4 = np.zeros((n_cores, P, nblk, H), np.float32)
    dinv_pc[core_of_n, node_lane, slot_of_n] = dinv
    bat_pc[core_of_n, node_lane, slot_of_n] = batch.astype(np.float32)
    xp4[core_of_n, node_lane, slot_of_n] = xf
    xp_pc = xp4.reshape(n_cores, P, nblk * H)

    return dict(eidx16=eidx16, edst=edst, dinv=dinv_pc, batg=bat_pc, xp=xp_pc,
                npc=npc, nblk=nblk, nsg=nsg, sgblk=sgblk, ntok=ntok,
                nchunk=nchunk, Kmax=Kmax, chunk_pos=chunk_pos,
                sg_tok0=sg_tok0, sg_w_ranges=sg_w_ranges, wrows=wrows, H=H)


# --------------------------------------------------------------------------
# Device kernel builder
# --------------------------------------------------------------------------

def build_nc(pp, G, C, n_cores=NCORES):
    """Build the Bass program (shared SPMD across n_cores)."""
    import concourse.bacc as bacc
    import concourse.mybir as mybir
    import concourse.tile as tile
    from contextlib import ExitStack

    H = pp["H"]
    nblk, nsg, ntok, nchunk = pp["nblk"], pp["nsg"], pp["ntok"], pp["nchunk"]
    Kmax, chunk_pos = pp["Kmax"], pp["chunk_pos"]
    sg_tok0, sg_w_ranges, wrows = pp["sg_tok0"], pp["sg_w_ranges"], pp["wrows"]
    RG = [list(range(n_cores))]
    EL = P  # padded feature width (256B rows)

    f32, bf16 = mybir.dt.float32, mybir.dt.bfloat16
    i16 = mybir.dt.int16
    AL = mybir.AluOpType

    nc = bacc.Bacc("TRN2", target_bir_lowering=False, debug=False,
                   enable_asserts=False, num_devices=n_cores,
                   num_swdge_queues=2)

    eidx_d = nc.dram_tensor("eidx", [16, ntok // 16], i16, kind="ExternalInput")
    edst_d = nc.dram_tensor("edst", [P, nchunk], bf16, kind="ExternalInput")
    xp_d = nc.dram_tensor("xp", [P, nblk * H], f32, kind="ExternalInput")
    dinv_d = nc.dram_tensor("dinv", [P, nblk], f32, kind="ExternalInput")
    batg_d = nc.dram_tensor("batg", [P, nblk], f32, kind="ExternalInput")
    iota_bf_d = nc.dram_tensor("iota_bf", [P, P], bf16, kind="ExternalInput")
    ident_d = nc.dram_tensor("ident", [P, P], bf16, kind="ExternalInput")
    iota_f_d = nc.dram_tensor("iota_f", [P, P], f32, kind="ExternalInput")
    w_d = [nc.dram_tensor(f"w{l}", [H, H], f32, kind="ExternalInput")
           for l in range(3)]
    bias_d = [nc.dram_tensor(f"bias{l}", [P, H], f32, kind="ExternalInput")
              for l in range(3)]
    wl_d = nc.dram_tensor("wl", [H, C], f32, kind="ExternalInput")
    biasl_d = nc.dram_tensor("biasl", [P, C], f32, kind="ExternalInput")
    cinv_d = nc.dram_tensor("cinv", [P, 1], f32, kind="ExternalInput")
    out_d = nc.dram_tensor("out", [G, C], f32, kind="ExternalOutput")

    with tile.TileContext(nc) as tc:
        with ExitStack() as ctx:
            const = ctx.enter_context(tc.tile_pool(name="const", bufs=1))
            msgs_tp = ctx.enter_context(tc.tile_pool(name="msgs", bufs=2))
            m_tp = ctx.enter_context(tc.tile_pool(name="mb", bufs=3))
            s_tp = ctx.enter_context(tc.tile_pool(name="st", bufs=3))
            e_tp = ctx.enter_context(tc.tile_pool(name="ep", bufs=4))
            agg_ps = ctx.enter_context(tc.tile_pool(name="aggp", bufs=4,
                                                    space="PSUM"))
            out_ps = ctx.enter_context(tc.tile_pool(name="outp", bufs=2,
                                                    space="PSUM"))
            fin_ps = ctx.enter_context(tc.tile_pool(name="finp", bufs=1,
                                                    space="PSUM"))
            dram = ctx.enter_context(tc.tile_pool(name="dram", bufs=1,
                                                  space="DRAM"))

            eidx_sb = const.tile([128, ntok // 16], i16)
            edst_sb = const.tile([P, nchunk], bf16)
            iota_bf = const.tile([P, P], bf16)
            ident_sb = const.tile([P, P], bf16)
            iota_f = const.tile([P, P], f32)
            dinv_sb = const.tile([P, nblk], f32)
            batg_sb = const.tile([P, nblk], f32)
            w_sb = [const.tile([H, H], f32, tag=f"w{l}", name=f"w{l}_sb")
                    for l in range(3)]
            bias_sb = [const.tile([P, H], f32, tag=f"b{l}", name=f"b{l}_sb")
                       for l in range(3)]
            wl_sb = const.tile([H, C], f32)
            biasl_sb = const.tile([P, C], f32)
            cinv_sb = const.tile([P, 1], f32)
            ht_sb = const.tile([P, nblk, EL], bf16)   # h~ slice, 256B rows
            h3_sb = const.tile([P, nblk * H], f32)
            xp_sb = const.tile([P, nblk * H], f32)

            # idx tile: replicate the [16, S] wrap to all 8 partition groups
            for g8 in range(8):
                nc.sync.dma_start(eidx_sb[:][g8 * 16:(g8 + 1) * 16, :],
                                  eidx_d.ap())
            nc.sync.dma_start(edst_sb[:], edst_d.ap())
            nc.sync.dma_start(iota_bf[:], iota_bf_d.ap())
            nc.sync.dma_start(ident_sb[:], ident_d.ap())
            nc.sync.dma_start(iota_f[:], iota_f_d.ap())
            nc.sync.dma_start(dinv_sb[:], dinv_d.ap())
            nc.sync.dma_start(batg_sb[:], batg_d.ap())
            for l in range(3):
                nc.sync.dma_start(w_sb[l][:], w_d[l].ap())
                nc.sync.dma_start(bias_sb[l][:], bias_d[l].ap())
            nc.sync.dma_start(wl_sb[:], wl_d.ap())
            nc.sync.dma_start(biasl_sb[:], biasl_d.ap())
            nc.sync.dma_start(cinv_sb[:], cinv_d.ap())
            nc.sync.dma_start(xp_sb[:], xp_d.ap())
            # zero the padding feature columns of h~ once
            nc.vector.memset(ht_sb[:], 0.0)

            in_cc = dram.tile([P, nblk * EL], bf16)
            hfull = [dram.tile([n_cores * P, nblk * EL], bf16,
                               addr_space="Shared", tag=f"hfull{l}",
                               name=f"hfull{l}") for l in range(3)]
            prd_in = dram.tile([H, P], f32)
            prd_out = dram.tile([H, P], f32, addr_space="Shared")

            # layer-1 input: h~ = dinv * x (bf16) into 256B rows
            for bi in range(nblk):
                nc.vector.tensor_scalar(
                    out=ht_sb[:][:, bi, 0:H],
                    in0=xp_sb[:][:, bi * H:(bi + 1) * H],
                    scalar1=dinv_sb[:][:, bi:bi + 1], scalar2=None,
                    op0=AL.mult)

            for l in range(3):
                last = l == 2
                nc.sync.dma_start(in_cc[:],
                                  ht_sb[:].rearrange("p b e -> p (b e)"))
                nc.gpsimd.collective_compute(
                    "AllGather", AL.bypass, replica_groups=RG,
                    ins=[in_cc.opt()], outs=[hfull[l].opt()])
                gat = hfull[l][:].rearrange("p (b e) -> (p b) e", e=EL)

                call_no = 0
                for sg in range(nsg):
                    tok0 = sg_tok0[sg]
                    sg_ntok = (sg_w_ranges[sg][-1][2] - tok0)
                    msgs = msgs_tp.tile([P, sg_ntok // P, EL], bf16,
                                        tag="msgs", name="msgs")
                    for (ww, t0, t1) in sg_w_ranges[sg]:
                        src_win = gat[ww * wrows:(ww + 1) * wrows, :]
                        t = t0
                        while t < t1:
                            tc_ = min(TMAX, t1 - t)
                            nc.gpsimd.dma_gather(
                                out_ap=msgs[:][:, (t - tok0) // P:
                                               (t - tok0 + tc_) // P, :],
                                in_ap=src_win,
                                idxs_ap=eidx_sb[:][:, t // 16:(t + tc_) // 16],
                                num_idxs=tc_, num_idxs_reg=tc_,
                                elem_size=EL, queue_num=call_no % 2)
                            call_no += 1
                            t += tc_
                    for bi in range(sg * pp["sgblk"], (sg + 1) * pp["sgblk"]):
                        aggT = agg_ps.tile([H, P], f32, tag="agg", name="agg")
                        nmm = int(Kmax[bi].sum())
                        # self-loop term: aggT += ht_block.T @ I
                        nc.tensor.matmul(
                            aggT[:], lhsT=ht_sb[:][:, bi, 0:H],
                            rhs=ident_sb[:], start=True, stop=(nmm == 0))
                        imm = 0
                        for ww in range(4):
                            kb = int(Kmax[bi, ww])
                            if kb == 0:
                                continue
                            col = int(chunk_pos[bi, ww])
                            MB = m_tp.tile([P, kb * P], bf16, tag="MB",
                                           name="MB")
                            nc.vector.tensor_tensor(
                                out=MB[:].rearrange("p (c q) -> p c q", q=P),
                                in0=edst_sb[:][:, col:col + kb]
                                    .to_broadcast([P, kb, P]),
                                in1=iota_bf[:][:, None, :]
                                    .to_broadcast([P, kb, P]),
                                op=AL.is_equal)
                            for j in range(kb):
                                mc = col + j - tok0 // P
                                nc.tensor.matmul(
                                    aggT[:],
                                    lhsT=msgs[:][:, mc, 0:H],
                                    rhs=MB[:][:, j * P:(j + 1) * P],
                                    start=False, stop=(imm == nmm - 1))
                                imm += 1
                        sT = s_tp.tile([H, P], f32, tag="sT", name="sT")
                        nc.scalar.copy(out=sT[:], in_=aggT[:])
                        outb = out_ps.tile([P, H], f32, tag="outb", name="outb")
                        nc.tensor.matmul(outb[:], lhsT=sT[:], rhs=w_sb[l][:],
                                         start=True, stop=True)
                        dcol = dinv_sb[:][:, bi:bi + 1]
                        t1_ = e_tp.tile([P, H], f32, tag="t1", name="t1")
                        nc.vector.tensor_scalar(
                            out=t1_[:], in0=outb[:], scalar1=dcol,
                            scalar2=None, op0=AL.mult)
                        if not last:
                            t2 = e_tp.tile([P, H], f32, tag="t2", name="t2")
                            nc.vector.tensor_tensor(
                                out=t2[:], in0=t1_[:], in1=bias_sb[l][:],
                                op=AL.add)
                            nc.vector.tensor_scalar(
                                out=ht_sb[:][:, bi, 0:H], in0=t2[:],
                                scalar1=0.0, scalar2=dcol,
                                op0=AL.max, op1=AL.mult)
                        else:
                            nc.vector.tensor_tensor(
                                out=h3_sb[:][:, bi * H:(bi + 1) * H],
                                in0=t1_[:], in1=bias_sb[l][:], op=AL.add)

            # pooling: poolT[f, g] = sum_n h3[n, f] * (batch[n] == g)
            poolT = fin_ps.tile([H, P], f32, tag="poolT")
            for bi in range(nblk):
                Mg = m_tp.tile([P, P], f32, tag="Mg", name="Mg")
                nc.vector.tensor_scalar(
                    out=Mg[:], in0=iota_f[:],
                    scalar1=batg_sb[:][:, bi:bi + 1], scalar2=None,
                    op0=AL.is_equal)
                nc.tensor.matmul(poolT[:],
                                 lhsT=h3_sb[:][:, bi * H:(bi + 1) * H],
                                 rhs=Mg[:], start=(bi == 0),
                                 stop=(bi == nblk - 1))
            poolT_sb = s_tp.tile([H, P], f32, tag="poolTs")
            nc.vector.tensor_copy(out=poolT_sb[:], in_=poolT[:])
            nc.sync.dma_start(prd_in[:], poolT_sb[:])
            nc.gpsimd.collective_compute(
                "AllReduce", AL.add, replica_groups=RG,
                ins=[prd_in.opt()], outs=[prd_out.opt()])
            poolF = s_tp.tile([H, P], f32, tag="poolF")
            nc.sync.dma_start(poolF[:], prd_out[:])
            fin = fin_ps.tile([P, C], f32, tag="fin")
            nc.tensor.matmul(fin[:], lhsT=poolF[:], rhs=wl_sb[:],
                             start=True, stop=True)
            outf = e_tp.tile([P, C], f32, tag="outf")
            nc.vector.tensor_scalar(out=outf[:], in0=fin[:],
                                    scalar1=cinv_sb[:], scalar2=None,
                                    op0=AL.mult)
            outf2 = e_tp.tile([P, C], f32, tag="outf2")
            nc.vector.tensor_tensor(out=outf2[:], in0=outf[:],
                                    in1=biasl_sb[:], op=AL.add)
            nc.sync.dma_start(out_d.ap()[:, :], outf2[:][:G, :])

    nc.compile()
    return nc


def make_in_maps(pp, weights, G, n_cores=NCORES):
    W1, b1, W2, b2, W3, b3, Wl, bl, counts = weights
    H = pp["H"]
    C = np.asarray(Wl).shape[1]
    bf = ml_dtypes.bfloat16
    iota_row = np.arange(P, dtype=np.float32)
    iota_bf = np.ascontiguousarray(np.broadcast_to(iota_row, (P, P))).astype(bf)
    iota_f = np.ascontiguousarray(np.broadcast_to(iota_row, (P, P)))
    cinv = np.ones((P, 1), np.float32)
    cinv[:G, 0] = 1.0 / np.maximum(counts, 1.0)
    shared = {
        "iota_bf": iota_bf, "iota_f": iota_f,
        "ident": np.eye(P, dtype=np.float32).astype(bf),
        "w0": np.asarray(W1, np.float32), "w1": np.asarray(W2, np.float32),
        "w2": np.asarray(W3, np.float32),
        "bias0": np.ascontiguousarray(np.broadcast_to(b1, (P, H))).astype(np.float32),
        "bias1": np.ascontiguousarray(np.broadcast_to(b2, (P, H))).astype(np.float32),
        "bias2": np.ascontiguousarray(np.broadcast_to(b3, (P, H))).astype(np.float32),
        "wl": np.asarray(Wl, np.float32),
        "biasl": np.ascontiguousarray(np.broadcast_to(bl, (P, C))).astype(np.float32),
        "cinv": cinv,
    }
    maps = []
    for c in range(n_cores):
        m = dict(shared)
        m["eidx"] = pp["eidx16"][c]
        m["edst"] = pp["edst"][c].astype(bf)
        m["xp"] = pp["xp"][c]
        m["dinv"] = pp["dinv"][c]
        m["batg"] = pp["batg"][c]
        maps.append(m)
    return maps


LAST_RESULT = None
LAST_NC = None
LAST_IN_MAPS = None


def kernel(x, edge_index, batch, W1, b1, W2, b2, W3, b3, Wl, bl, **run_kwargs):
    """Full-input entry point. Shards across 8 cores, runs on HW, gathers."""
    global LAST_RESULT, LAST_NC, LAST_IN_MAPS
    from concourse.bass_utils import run_bass_kernel_spmd

    x = np.asarray(x, np.float32)
    edge_index = np.asarray(edge_index)
    batch = np.asarray(batch)
    G = G_FULL
    C = np.asarray(Wl).shape[1]

    pp = preprocess(x, edge_index, batch)
    counts = np.bincount(batch.astype(np.int64), minlength=G).astype(np.float32)
    nc = build_nc(pp, G, C)
    in_maps = make_in_maps(pp, (W1, b1, W2, b2, W3, b3, Wl, bl, counts), G)
    res = run_bass_kernel_spmd(nc, in_maps, core_ids=list(range(NCORES)),
                               **run_kwargs)
    LAST_RESULT, LAST_NC, LAST_IN_MAPS = res, nc, in_maps
    return res.results[0]["out"].astype(np.float32)



# revision 15
# speedup vs baseline: 1.1126x; 1.1126x over previous
"""3-layer GCN + global mean pool + linear head on 8 Trainium2 NeuronCores.

Strategy (dst-sharded message passing):
  - GCN normalization factorizes: norm_e = dinv[src]*dinv[dst], so each conv
    layer is  h' = relu( dinv * ((Adj+I) @ (dinv * h)) @ W + b ).  Only pure
    row gather + segment-sum on device; diagonal scalings are per-node ops.
  - Nodes (and their in-edges, self-loops appended) are sharded across the 8
    cores by contiguous dst ranges.  Edges are grouped by (dst 128-block,
    source window) and padded to a uniform number of 128-edge chunks (padding
    uses dst_rel=-1, whose one-hot column is zero).
  - Per layer: each core row-scales its h slice (h~ = dinv*h, bf16, features
    padded to 128 so rows are 256B), slices are AllGathered into a DRAM
    table, and each core gathers its edges' source rows with dma_gather
    (int16 indices -> the table is split into 4 windows of 25088 rows; calls
    are <=1024 tokens to fit the SWDGE descriptor ring).  Segment-sum runs on
    the TensorEngine: per 128-edge chunk, aggT[64f,128d] += msgs[128e,0:64].T
    @ M[128e,128d], with M built by one broadcast is_equal against an iota
    tile.  The layer weight applies after aggregation (W commutes with the
    sum), then dinv/bias/relu as per-partition DVE ops.
  - Mean-pool uses the same one-hot matmul against graph ids, partials are
    AllReduced, and the head matmul finishes on every core.

Host-side work is sharding-style preprocessing only: edge sort/group/pad,
degree bincount (dinv), graph-size bincount, layout permutation.
"""
"""3-layer GCN + global mean pool + linear head on 8 Trainium2 NeuronCores.

Strategy (dst-sharded message passing):
  - GCN normalization factorizes: norm_e = dinv[src]*dinv[dst], so each conv
    layer is  h' = relu( dinv * ((Adj+I) @ (dinv * h)) @ W + b ).  Only pure
    row gather + segment-sum on device; diagonal scalings are per-node ops.
  - Nodes (and their in-edges, self-loops appended) are sharded across the 8
    cores by contiguous dst ranges.  Edges are grouped by (dst 128-block,
    source window) and padded to a uniform number of 128-edge chunks (padding
    uses dst_rel=-1, whose one-hot column is zero).
  - Per layer: each core row-scales its h slice (h~ = dinv*h, bf16, features
    padded to 128 so rows are 256B), slices are AllGathered into a DRAM
    table, and each core gathers its edges' source rows with dma_gather
    (int16 indices -> the table is split into 4 windows of 25088 rows; calls
    are <=1024 tokens to fit the SWDGE descriptor ring).  Segment-sum runs on
    the TensorEngine: per 128-edge chunk, aggT[64f,128d] += msgs[128e,0:64].T
    @ M[128e,128d], with M built by one broadcast is_equal against an iota
    tile.  The layer weight applies after aggregation (W commutes with the
    sum), then dinv/bias/relu as per-partition DVE ops.
  - Mean-pool uses the same one-hot matmul against graph ids, partials are
    AllReduced, and the head matmul finishes on every core.

Host-side work is sharding-style preprocessing only: edge sort/group/pad,
degree bincount (dinv), graph-size bincount, layout permutation.
"""
import numpy as np
import ml_dtypes

P = 128
NCORES = 8
NWIN = 4          # int16 source windows
TMAX = 1024       # max tokens per dma_gather call (SWDGE ring limit)
SGBLK = 7         # dst blocks per super-group (msgs buffer granularity)

# Full-size problem dims (nn_GCN_13881334300836)
N_FULL, E_FULL, D_FULL, C_FULL, G_FULL = 100_000, 1_250_000, 64, 10, 128


# --------------------------------------------------------------------------
# Host preprocessing
# --------------------------------------------------------------------------

def preprocess(x, edge_index, batch, n_cores=NCORES):
    """Shard nodes/edges; build window-grouped, chunk-padded gather indices.

    Permuted global row for node n (core, local=n-core*npc, b=local//P,
    p=local%P):  row = (core*P + p)*nblk + b.  Each core's h~ slice is then
    one contiguous SBUF->DRAM DMA and AllGather concatenation.  Window w
    covers rows [w*wrows, (w+1)*wrows), wrows = n_cores*P*nblk/NWIN.
    """
    N, H = x.shape
    assert N % n_cores == 0
    npc = N // n_cores
    nblk = (npc + P - 1) // P
    npad = nblk * P
    wrows = n_cores * npad // NWIN
    assert wrows <= 32768

    ei = edge_index.astype(np.int64)
    src_all = ei[0]
    dst_all = ei[1]

    # degree includes the self-loop (gcn_norm); loops are NOT streamed --
    # their term is one identity matmul per block on-chip.
    deg = (np.bincount(dst_all, minlength=N) + 1).astype(np.float32)
    dinv = (1.0 / np.sqrt(np.maximum(deg, 1.0))).astype(np.float32)

    # serpentine deal by in-degree -> balanced (core, slot) bins
    nbins = n_cores * nblk
    order = np.argsort(-deg, kind="stable")
    idx = np.arange(N)
    rr = idx // nbins
    kk = idx % nbins
    binidx = np.where(rr % 2 == 0, kk, nbins - 1 - kk)
    node_bin = np.empty(N, np.int64)
    node_lane = np.empty(N, np.int64)
    node_bin[order] = binidx
    node_lane[order] = rr
    core_of_n = node_bin % n_cores
    slot_of_n = node_bin // n_cores
    perm_n = (core_of_n * P + node_lane) * nblk + slot_of_n
    src_perm = perm_n[src_all]
    win_all = src_perm // wrows
    e_core = core_of_n[dst_all]
    e_slot = slot_of_n[dst_all]
    e_lane = node_lane[dst_all]

    # per-core (block, window) edge lists
    edges = []   # [core][b][w] -> (idx int16 array, drel array)
    K = np.zeros((n_cores, nblk, NWIN), np.int64)
    for c in range(n_cores):
        m = e_core == c
        s = src_perm[m]
        w = win_all[m]
        dslot = e_slot[m]
        dlane = e_lane[m]
        key = dslot * NWIN + w
        o2 = np.argsort(key, kind="stable")
        s, w, key = s[o2], w[o2], key[o2]
        dlane = dlane[o2]
        cnt = np.bincount(key, minlength=nblk * NWIN)
        off = np.zeros(nblk * NWIN + 1, np.int64)
        np.cumsum(cnt, out=off[1:])
        percore = []
        for b in range(nblk):
            row = []
            for ww in range(NWIN):
                k = b * NWIN + ww
                sl = slice(off[k], off[k + 1])
                row.append(((s[sl] - ww * wrows).astype(np.int16),
                            dlane[sl].astype(np.float32)))
                K[c, b, ww] = (cnt[k] + P - 1) // P
            percore.append(row)
        edges.append(percore)

    Kmax = K.max(axis=0)          # [nblk, NWIN] uniform chunk counts
    sgblk = SGBLK
    if nblk % sgblk != 0:
        sgblk = next((g for g in (7, 8, 6, 5, 4, 9, 10, 3, 2) if nblk % g == 0),
                     nblk)
    nsg = nblk // sgblk

    # token stream: for sg: for w: for b in sg: Kmax[b,w] chunks of 128
    # chunk positions (global column index) and per-(sg,w) call splits
    chunk_pos = np.zeros((nblk, NWIN), np.int64)   # starting chunk column
    sg_tok0 = []                                   # sg -> token start
    sg_w_ranges = []                               # sg -> [(w, tok0, tok1)]
    pos = 0
    for sg in range(nsg):
        sg_tok0.append(pos * P)
        rngs = []
        for ww in range(NWIN):
            t0 = pos * P
            for b in range(sg * sgblk, (sg + 1) * sgblk):
                chunk_pos[b, ww] = pos
                pos += Kmax[b, ww]
            rngs.append((ww, t0, pos * P))
        sg_w_ranges.append(rngs)
    nchunk = pos
    ntok = nchunk * P

    eidx16 = np.zeros((n_cores, 16, ntok // 16), np.int16)
    edst = np.full((n_cores, P, nchunk), -1.0, np.float32)
    for c in range(n_cores):
        stream = np.zeros(ntok, np.int16)
        for b in range(nblk):
            for ww in range(NWIN):
                s16, dr = edges[c][b][ww]
                t0 = chunk_pos[b, ww] * P
                stream[t0:t0 + len(s16)] = s16
                # dst_rel per slot: token t -> (p=t%128, chunk=t//128)
                col = chunk_pos[b, ww]
                nchunks_b = Kmax[b, ww]
                dcols = np.full((nchunks_b * P,), -1.0, np.float32)
                dcols[:len(dr)] = dr
                edst[c][:, col:col + nchunks_b] = dcols.reshape(nchunks_b, P).T
        eidx16[c] = stream.reshape(ntok // 16, 16).T

    dinv_pc = np.zeros((n_cores, P, nblk), np.float32)
    bat_pc = np.full((n_cores, P, nblk), -1.0, np.float32)
    xf = np.asarray(x, np.float32)
    xp4 = np.zeros((n_cores, P, nblk, H), np.float32)
    dinv_pc[core_of_n, node_lane, slot_of_n] = dinv
    bat_pc[core_of_n, node_lane, slot_of_n] = batch.astype(np.float32)
    xp4[core_of_n, node_lane, slot_of_n] = xf
    xp_pc = xp4.reshape(n_cores, P, nblk * H)

    kbmax = int(Kmax.max())

    return dict(eidx16=eidx16, edst=edst, dinv=dinv_pc, batg=bat_pc, xp=xp_pc,
                npc=npc, nblk=nblk, nsg=nsg, sgblk=sgblk, ntok=ntok,
                nchunk=nchunk, Kmax=Kmax, chunk_pos=chunk_pos, kbmax=kbmax,
                sg_tok0=sg_tok0, sg_w_ranges=sg_w_ranges, wrows=wrows, H=H)


# --------------------------------------------------------------------------
# Device kernel builder
# --------------------------------------------------------------------------

def build_nc(pp, G, C, n_cores=NCORES, ablate_cc=False, ablate_gather=False,
             n_queues=2):
    """Build the Bass program (shared SPMD across n_cores).

    ablate_cc / ablate_gather build timing-ablation variants (numerically
    wrong): local copies instead of AllGather / skipped dma_gather calls.
    """
    import concourse.bacc as bacc
    import concourse.mybir as mybir
    import concourse.tile as tile
    from contextlib import ExitStack

    H = pp["H"]
    nblk, nsg, ntok, nchunk = pp["nblk"], pp["nsg"], pp["ntok"], pp["nchunk"]
    Kmax, chunk_pos = pp["Kmax"], pp["chunk_pos"]
    sg_tok0, sg_w_ranges, wrows = pp["sg_tok0"], pp["sg_w_ranges"], pp["wrows"]
    RG = [list(range(n_cores))]
    EL = P  # padded feature width (256B rows)

    f32, bf16 = mybir.dt.float32, mybir.dt.bfloat16
    i16 = mybir.dt.int16
    AL = mybir.AluOpType

    nc = bacc.Bacc("TRN2", target_bir_lowering=False, debug=False,
                   enable_asserts=False, num_devices=n_cores,
                   num_swdge_queues=n_queues)

    kbmax = pp["kbmax"]
    eidx_d = nc.dram_tensor("eidx", [16, ntok // 16], i16, kind="ExternalInput")
    edst_d = nc.dram_tensor("edst", [P, nchunk], bf16, kind="ExternalInput")
    xp_d = nc.dram_tensor("xp", [P, nblk * H], f32, kind="ExternalInput")
    dinv_d = nc.dram_tensor("dinv", [P, nblk], f32, kind="ExternalInput")
    batg_d = nc.dram_tensor("batg", [P, nblk], f32, kind="ExternalInput")
    iota_w_d = nc.dram_tensor("iota_w", [P, P * kbmax], bf16,
                              kind="ExternalInput")
    ident_d = nc.dram_tensor("ident", [P, P], bf16, kind="ExternalInput")
    iota_f_d = nc.dram_tensor("iota_f", [P, P], f32, kind="ExternalInput")
    w_d = [nc.dram_tensor(f"w{l}", [H, H], f32, kind="ExternalInput")
           for l in range(3)]
    bias_d = [nc.dram_tensor(f"bias{l}", [P, H], f32, kind="ExternalInput")
              for l in range(3)]
    wl_d = nc.dram_tensor("wl", [H, C], f32, kind="ExternalInput")
    biasl_d = nc.dram_tensor("biasl", [P, C], f32, kind="ExternalInput")
    cinv_d = nc.dram_tensor("cinv", [P, 1], f32, kind="ExternalInput")
    out_d = nc.dram_tensor("out", [G, C], f32, kind="ExternalOutput")

    with tile.TileContext(nc) as tc:
        with ExitStack() as ctx:
            const = ctx.enter_context(tc.tile_pool(name="const", bufs=1))
            msgs_tp = ctx.enter_context(tc.tile_pool(name="msgs", bufs=2))
            m_tp = ctx.enter_context(tc.tile_pool(name="mb", bufs=3))
            s_tp = ctx.enter_context(tc.tile_pool(name="st", bufs=3))
            e_tp = ctx.enter_context(tc.tile_pool(name="ep", bufs=4))
            agg_ps = ctx.enter_context(tc.tile_pool(name="aggp", bufs=4,
                                                    space="PSUM"))
            out_ps = ctx.enter_context(tc.tile_pool(name="outp", bufs=2,
                                                    space="PSUM"))
            fin_ps = ctx.enter_context(tc.tile_pool(name="finp", bufs=1,
                                                    space="PSUM"))
            dram = ctx.enter_context(tc.tile_pool(name="dram", bufs=1,
                                                  space="DRAM"))

            eidx_sb = const.tile([128, ntok // 16], i16)
            edst_sb = const.tile([P, nchunk], bf16)
            iota_w = const.tile([P, P, kbmax], bf16)
            ident_sb = const.tile([P, P], bf16)
            iota_f = const.tile([P, P], f32)
            dinv_sb = const.tile([P, nblk], f32)
            batg_sb = const.tile([P, nblk], f32)
            w_sb = [const.tile([H, H], f32, tag=f"w{l}", name=f"w{l}_sb")
                    for l in range(3)]
            bias_sb = [const.tile([P, H], f32, tag=f"b{l}", name=f"b{l}_sb")
                       for l in range(3)]
            wl_sb = const.tile([H, C], f32)
            biasl_sb = const.tile([P, C], f32)
            cinv_sb = const.tile([P, 1], f32)
            ht_sb = const.tile([P, nblk, EL], bf16)   # h~ slice, 256B rows
            h3_sb = const.tile([P, nblk * H], f32)
            xp_sb = const.tile([P, nblk * H], f32)

            # idx tile: replicate the [16, S] wrap to all 8 partition groups
            for g8 in range(8):
                nc.sync.dma_start(eidx_sb[:][g8 * 16:(g8 + 1) * 16, :],
                                  eidx_d.ap())
            nc.sync.dma_start(edst_sb[:], edst_d.ap())
            nc.sync.dma_start(iota_w[:].rearrange("p q c -> p (q c)"),
                              iota_w_d.ap())
            nc.sync.dma_start(ident_sb[:], ident_d.ap())
            nc.sync.dma_start(iota_f[:], iota_f_d.ap())
            nc.sync.dma_start(dinv_sb[:], dinv_d.ap())
            nc.sync.dma_start(batg_sb[:], batg_d.ap())
            for l in range(3):
                nc.sync.dma_start(w_sb[l][:], w_d[l].ap())
                nc.sync.dma_start(bias_sb[l][:], bias_d[l].ap())
            nc.sync.dma_start(wl_sb[:], wl_d.ap())
            nc.sync.dma_start(biasl_sb[:], biasl_d.ap())
            nc.sync.dma_start(cinv_sb[:], cinv_d.ap())
            nc.sync.dma_start(xp_sb[:], xp_d.ap())
            # zero the padding feature columns of h~ once
            nc.vector.memset(ht_sb[:], 0.0)

            in_cc = dram.tile([P, nblk * EL], bf16)
            hf_space = {} if ablate_cc else dict(addr_space="Shared")
            hfull = [dram.tile([n_cores * P, nblk * EL], bf16,
                               tag=f"hfull{l}", name=f"hfull{l}",
                               **hf_space) for l in range(3)]
            prd_in = dram.tile([H, P], f32)
            prd_out = dram.tile([H, P], f32, addr_space="Shared")

            # layer-1 input: h~ = dinv * x (bf16) into 256B rows
            for bi in range(nblk):
                nc.vector.tensor_scalar(
                    out=ht_sb[:][:, bi, 0:H],
                    in0=xp_sb[:][:, bi * H:(bi + 1) * H],
                    scalar1=dinv_sb[:][:, bi:bi + 1], scalar2=None,
                    op0=AL.mult)

            for l in range(3):
                last = l == 2
                nc.sync.dma_start(in_cc[:],
                                  ht_sb[:].rearrange("p b e -> p (b e)"))
                if ablate_cc:
                    # timing ablation: same bytes landed locally, no CC
                    for q in range(n_cores):
                        nc.sync.dma_start(
                            hfull[l][:][q * P:(q + 1) * P, :], in_cc[:])
                else:
                    nc.gpsimd.collective_compute(
                        "AllGather", AL.bypass, replica_groups=RG,
                        ins=[in_cc.opt()], outs=[hfull[l].opt()])
                gat = hfull[l][:].rearrange("p (b e) -> (p b) e", e=EL)

                call_no = 0
                for sg in range(nsg):
                    tok0 = sg_tok0[sg]
                    sg_ntok = (sg_w_ranges[sg][-1][2] - tok0)
                    msgs = msgs_tp.tile([P, sg_ntok // P, EL], bf16,
                                        tag="msgs", name="msgs")
                    if ablate_gather:
                        nc.vector.memset(msgs[:][:, 0:1, :], 0.0)
                    for (ww, t0, t1) in sg_w_ranges[sg]:
                        src_win = gat[ww * wrows:(ww + 1) * wrows, :]
                        t = t0
                        while t < t1:
                            tc_ = min(TMAX, t1 - t)
                            if not ablate_gather:
                                nc.gpsimd.dma_gather(
                                    out_ap=msgs[:][:, (t - tok0) // P:
                                                   (t - tok0 + tc_) // P, :],
                                    in_ap=src_win,
                                    idxs_ap=eidx_sb[:][:, t // 16:
                                                       (t + tc_) // 16],
                                    num_idxs=tc_, num_idxs_reg=tc_,
                                    elem_size=EL, queue_num=call_no % n_queues)
                            call_no += 1
                            t += tc_
                    for bi in range(sg * pp["sgblk"], (sg + 1) * pp["sgblk"]):
                        aggT = agg_ps.tile([H, P], f32, tag="agg", name="agg")
                        nmm = int(Kmax[bi].sum())
                        # self-loop term: aggT += ht_block.T @ I
                        nc.tensor.matmul(
                            aggT[:], lhsT=ht_sb[:][:, bi, 0:H],
                            rhs=ident_sb[:], start=True, stop=(nmm == 0))
                        imm = 0
                        for ww in range(4):
                            kb = int(Kmax[bi, ww])
                            if kb == 0:
                                continue
                            col = int(chunk_pos[bi, ww])
                            MB = m_tp.tile([P, P, kb], bf16, tag="MB",
                                           name="MB")
                            nc.vector.tensor_tensor(
                                out=MB[:],
                                in0=edst_sb[:][:, None, col:col + kb]
                                    .to_broadcast([P, P, kb]),
                                in1=iota_w[:][:, :, 0:kb],
                                op=AL.is_equal)
                            for j in range(kb):
                                mc = col + j - tok0 // P
                                nc.tensor.matmul(
                                    aggT[:],
                                    lhsT=msgs[:][:, mc, 0:H],
                                    rhs=MB[:][:, :, j],
                                    start=False, stop=(imm == nmm - 1))
                                imm += 1
                        sT = s_tp.tile([H, P], f32, tag="sT", name="sT")
                        nc.scalar.copy(out=sT[:], in_=aggT[:])
                        outb = out_ps.tile([P, H], f32, tag="outb", name="outb")
                        nc.tensor.matmul(outb[:], lhsT=sT[:], rhs=w_sb[l][:],
                                         start=True, stop=True)
                        dcol = dinv_sb[:][:, bi:bi + 1]
                        t1_ = e_tp.tile([P, H], f32, tag="t1", name="t1")
                        nc.vector.tensor_scalar(
                            out=t1_[:], in0=outb[:], scalar1=dcol,
                            scalar2=None, op0=AL.mult)
                        if not last:
                            t2 = e_tp.tile([P, H], f32, tag="t2", name="t2")
                            nc.vector.tensor_tensor(
                                out=t2[:], in0=t1_[:], in1=bias_sb[l][:],
                                op=AL.add)
                            nc.vector.tensor_scalar(
                                out=ht_sb[:][:, bi, 0:H], in0=t2[:],
                                scalar1=0.0, scalar2=dcol,
                                op0=AL.max, op1=AL.mult)
                        else:
                            nc.vector.tensor_tensor(
                                out=h3_sb[:][:, bi * H:(bi + 1) * H],
                                in0=t1_[:], in1=bias_sb[l][:], op=AL.add)

            # pooling: poolT[f, g] = sum_n h3[n, f] * (batch[n] == g)
            poolT = fin_ps.tile([H, P], f32, tag="poolT")
            for bi in range(nblk):
                Mg = m_tp.tile([P, P], f32, tag="Mg", name="Mg")
                nc.vector.tensor_scalar(
                    out=Mg[:], in0=iota_f[:],
                    scalar1=batg_sb[:][:, bi:bi + 1], scalar2=None,
                    op0=AL.is_equal)
                nc.tensor.matmul(poolT[:],
                                 lhsT=h3_sb[:][:, bi * H:(bi + 1) * H],
                                 rhs=Mg[:], start=(bi == 0),
                                 stop=(bi == nblk - 1))
            poolT_sb = s_tp.tile([H, P], f32, tag="poolTs")
            nc.vector.tensor_copy(out=poolT_sb[:], in_=poolT[:])
            nc.sync.dma_start(prd_in[:], poolT_sb[:])
            nc.gpsimd.collective_compute(
                "AllReduce", AL.add, replica_groups=RG,
                ins=[prd_in.opt()], outs=[prd_out.opt()])
            poolF = s_tp.tile([H, P], f32, tag="poolF")
            nc.sync.dma_start(poolF[:], prd_out[:])
            fin = fin_ps.tile([P, C], f32, tag="fin")
            nc.tensor.matmul(fin[:], lhsT=poolF[:], rhs=wl_sb[:],
                             start=True, stop=True)
            outf = e_tp.tile([P, C], f32, tag="outf")
            nc.vector.tensor_scalar(out=outf[:], in0=fin[:],
                                    scalar1=cinv_sb[:], scalar2=None,
                                    op0=AL.mult)
            outf2 = e_tp.tile([P, C], f32, tag="outf2")
            nc.vector.tensor_tensor(out=outf2[:], in0=outf[:],
                                    in1=biasl_sb[:], op=AL.add)
            nc.sync.dma_start(out_d.ap()[:, :], outf2[:][:G, :])

    nc.compile()
    return nc


def make_in_maps(pp, weights, G, n_cores=NCORES):
    W1, b1, W2, b2, W3, b3, Wl, bl, counts = weights
    H = pp["H"]
    C = np.asarray(Wl).shape[1]
    bf = ml_dtypes.bfloat16
    kbmax = pp["kbmax"]
    iota_row = np.arange(P, dtype=np.float32)
    iota_w = np.ascontiguousarray(np.broadcast_to(
        iota_row[None, :, None], (P, P, kbmax))).reshape(P, P * kbmax).astype(bf)
    iota_f = np.ascontiguousarray(np.broadcast_to(iota_row, (P, P)))
    cinv = np.ones((P, 1), np.float32)
    cinv[:G, 0] = 1.0 / np.maximum(counts, 1.0)
    shared = {
        "iota_w": iota_w, "iota_f": iota_f,
        "ident": np.eye(P, dtype=np.float32).astype(bf),
        "w0": np.asarray(W1, np.float32), "w1": np.asarray(W2, np.float32),
        "w2": np.asarray(W3, np.float32),
        "bias0": np.ascontiguousarray(np.broadcast_to(b1, (P, H))).astype(np.float32),
        "bias1": np.ascontiguousarray(np.broadcast_to(b2, (P, H))).astype(np.float32),
        "bias2": np.ascontiguousarray(np.broadcast_to(b3, (P, H))).astype(np.float32),
        "wl": np.asarray(Wl, np.float32),
        "biasl": np.ascontiguousarray(np.broadcast_to(bl, (P, C))).astype(np.float32),
        "cinv": cinv,
    }
    maps = []
    for c in range(n_cores):
        m = dict(shared)
        m["eidx"] = pp["eidx16"][c]
        m["edst"] = pp["edst"][c].astype(bf)
        m["xp"] = pp["xp"][c]
        m["dinv"] = pp["dinv"][c]
        m["batg"] = pp["batg"][c]
        maps.append(m)
    return maps


LAST_RESULT = None
LAST_NC = None
LAST_IN_MAPS = None


def kernel(x, edge_index, batch, W1, b1, W2, b2, W3, b3, Wl, bl, **run_kwargs):
    """Full-input entry point. Shards across 8 cores, runs on HW, gathers."""
    global LAST_RESULT, LAST_NC, LAST_IN_MAPS
    from concourse.bass_utils import run_bass_kernel_spmd

    x = np.asarray(x, np.float32)
    edge_index = np.asarray(edge_index)
    batch = np.asarray(batch)
    G = G_FULL
    C = np.asarray(Wl).shape[1]

    pp = preprocess(x, edge_index, batch)
    counts = np.bincount(batch.astype(np.int64), minlength=G).astype(np.float32)
    nc = build_nc(pp, G, C, n_queues=4)
    in_maps = make_in_maps(pp, (W1, b1, W2, b2, W3, b3, Wl, bl, counts), G)
    res = run_bass_kernel_spmd(nc, in_maps, core_ids=list(range(NCORES)),
                               **run_kwargs)
    LAST_RESULT, LAST_NC, LAST_IN_MAPS = res, nc, in_maps
    return res.results[0]["out"].astype(np.float32)

